# revision 6
# baseline (speedup 1.0000x reference)
"""EnhancedDynamicChannelAttention Trainium2 kernel (bf16 pipeline).

Reference computation (B=16, S=2048, C=1024, H=8, HD=128):
    q[b,h,:]   = pref[b,h]*Wq[:,0] + bq
    k          = f @ Wk.T + bk ;  v = f @ Wv.T + bv       (per head slice)
    scores     = softmax_s(q . k)                          [B,H,S]
    ctx[b,h,:] = sum_s scores * v[b,s,h,:]                 [B,H,HD]
    out        = f + broadcast_s(ctx)

Algebraic folding (exact up to fp reassociation):
  - softmax shift invariance  -> the q.bk term drops entirely.
  - scores[b,h,s] = f[b,s,h,:] . qk[b,h,:]  with  qk = (pref*Wq+bq) @ Wk
  - sum_s attn = 1  ->  ctx = Wv @ (sum_s attn*f[b,s,h,:]) + bv
  So k/v are never materialized.

Distribution: pure data parallel over batch, 2 batches per core.

v2 schedule (vs the 88.7us baseline):
  - qk rows ride ONE tiny [2,C] DMA; the [P,C] broadcasts are built
    on-chip (PE ones-outer-product -> PSUM -> ACT copy).  The old
    to_broadcast DMAs (4x256KB of 2KB descriptors) headed the ring and
    delayed the first f tile ~8us.
  - ones/onesrow/one1 tiles come from memsets, wvt/bvf/id8 from small
    scalar-ring DMAs; the sync ring carries ONLY f loads + stores.
  - f loads for both batches issue up front; steady state runs the ring
    at line rate.
  - residual adds split DVE/Pool; Pool also takes the last two score
    muls of each batch (its tiles' folds stay on DVE).
  - tails (T0/T1) are emitted in natural order between score phases;
    PSUM banks are reused sequentially (uwf bufs=1 still works because
    T0's PSUM copy precedes b1's first accumulation in the ACT queue).
"""

import numpy as np

B, S, C = 16, 2048, 1024
H, HD = 8, 128
N_CORES = 8
BPC = B // N_CORES          # batches per core
ST = 2                      # s-rows per partition in a super tile
P = 128
SUP = S // (P * ST)         # super tiles per batch (8)
NT = S // P                 # sub tiles per batch (16)

# score-mul tiles computed on the Pool engine
POOL_MULS = {0: (6, 7), 1: (6, 7)}
# residual tiles handed to Pool (rest on DVE)
POOL_RESID = {0: (5, 6, 7), 1: (6,)}

_CACHE = {}


def _build_program():
    import concourse.bass as bass
    import concourse.bacc as bacc
    import concourse.tile as tile
    from concourse import mybir

    f32 = mybir.dt.float32
    f16 = mybir.dt.float16
    bf16 = mybir.dt.bfloat16

    nc = bacc.Bacc("TRN2", debug=False, num_devices=N_CORES)
    f_in = nc.dram_tensor("features", [BPC, S, C], bf16, kind="ExternalInput")
    qk_in = nc.dram_tensor("qkflat", [BPC, C], bf16, kind="ExternalInput")
    wvt_in = nc.dram_tensor("wvt", [HD, HD], bf16, kind="ExternalInput")
    bvf_in = nc.dram_tensor("bvflat", [1, C], bf16, kind="ExternalInput")
    id8_in = nc.dram_tensor("ident8", [8, 8], f32, kind="ExternalInput")
    out_t = nc.dram_tensor("out", [BPC, S, C], bf16, kind="ExternalOutput")

    with tile.TileContext(nc) as tc:
        with (
            tc.tile_pool(name="fpool", bufs=BPC) as fpool,
            tc.tile_pool(name="tmppool", bufs=2) as tmppool,
            tc.tile_pool(name="ptmppool", bufs=2) as ptmppool,
            tc.tile_pool(name="spool", bufs=3) as spool,
            tc.tile_pool(name="small", bufs=2) as small,
            tc.tile_pool(name="ostage", bufs=3) as ostage,
            tc.tile_pool(name="ostagep", bufs=2) as ostagep,
            tc.tile_pool(name="singles", bufs=1) as singles,
            tc.tile_pool(name="ps_uwf", bufs=1, space="PSUM") as ps_uwf,
            tc.tile_pool(name="ps_tail", bufs=1, space="PSUM") as ps_tail,
        ):
            # ---- f loads first: they own the sync ring ----
            fbs = [None] * BPC

            def load_batch(b):
                fb = fpool.tile([P, NT, C], bf16, tag="fb")
                fbs[b] = fb
                fview = f_in[b].rearrange("(st p t) c -> st p t c", p=P, t=ST)
                for st in range(SUP):
                    lo = st * ST
                    nc.sync.dma_start(out=fb[:, lo : lo + ST, :], in_=fview[st])

            load_batch(0)

            # ---- tiny constants on the scalar ring ----
            qk_rows = []
            for b in range(BPC):
                qkr = singles.tile([1, C], bf16, tag=f"qkr{b}")
                nc.scalar.dma_start(out=qkr, in_=qk_in[b : b + 1, :])
                qk_rows.append(qkr)
            wvt_sb = singles.tile([HD, HD], bf16)
            nc.scalar.dma_start(out=wvt_sb, in_=wvt_in[:, :])
            bvf_sb = singles.tile([1, C], bf16)
            nc.scalar.dma_start(out=bvf_sb, in_=bvf_in[:, :])
            id8_sb = singles.tile([8, 8], f32)
            nc.scalar.dma_start(out=id8_sb, in_=id8_in[:, :])

            ones_sb = singles.tile([P, 1], bf16)
            nc.gpsimd.memset(ones_sb, 1.0)
            onesrow_sb = singles.tile([1, P], bf16)
            nc.gpsimd.memset(onesrow_sb, 1.0)
            one1_sb = singles.tile([1, 1], bf16)
            nc.gpsimd.memset(one1_sb, 1.0)

            # ---- on-chip qk broadcast: ones ⊗ qk_row -> PSUM -> bf16 SBUF ----
            qk_bcs = []
            qk_bcs_pool = []
            for b in range(BPC):
                qk_bc = small.tile([P, C], bf16, tag="qkbc")
                need_pool = bool(POOL_MULS[b] or POOL_RESID[b])
                if need_pool:
                    qk_bcp = small.tile([P, C], bf16, tag="qkbcp")
                else:
                    qk_bcp = None
                for half in range(2):
                    cs = slice(half * 512, (half + 1) * 512)
                    qkbc_ps = ps_tail.tile([P, 512], f32, tag="qkbcps")
                    nc.tensor.matmul(
                        qkbc_ps, onesrow_sb, qk_rows[b][:, cs],
                        start=True, stop=True,
                    )
                    nc.scalar.copy(out=qk_bc[:, cs], in_=qkbc_ps)
                    if need_pool:
                        nc.scalar.copy(out=qk_bcp[:, cs], in_=qkbc_ps)
                qk_bcs.append(qk_bc)
                qk_bcs_pool.append(qk_bcp)

            load_batch(1)

            uwfs = [None] * BPC
            sums = [None] * BPC
            recips = [None] * BPC
            ctxs = [None] * BPC
            sumE2 = ps_uwf.tile([8, BPC], f32, tag="sumE")

            def pool_mul_tile(b, st):
                fb = fbs[b]
                lo = st * ST
                qk_bc3p = qk_bcs_pool[b].rearrange(
                    "p (o c) -> p o c", o=1
                ).broadcast_to([P, ST, C])
                tmp = ptmppool.tile([P, ST, C], bf16, tag="tmpp")
                nc.gpsimd.tensor_mul(tmp, fb[:, lo : lo + ST, :], qk_bc3p)
                return tmp

            def scores_super_tile(b, st, qk_bc3, uwfA, uwfB, first, last,
                                  pool_tmp=None):
                fb = fbs[b]
                lo = st * ST
                if pool_tmp is not None:
                    tmp = pool_tmp
                else:
                    tmp = tmppool.tile([P, ST, C], bf16, tag="tmp")
                    nc.vector.tensor_mul(tmp, fb[:, lo : lo + ST, :], qk_bc3)
                # segmented reduce over d=128: two bf16 tree folds (DVE 2x)
                # + a short X reduce over 32
                tmpv = tmp.rearrange("p t (h d) -> p t h d", h=H)
                f64t = spool.tile([P, ST, H, 64], bf16, tag="fold64")
                nc.vector.tensor_add(f64t, tmpv[:, :, :, 0:64], tmpv[:, :, :, 64:128])
                f32t = spool.tile([P, ST, H, 32], bf16, tag="fold32")
                nc.vector.tensor_add(f32t, f64t[:, :, :, 0:32], f64t[:, :, :, 32:64])
                scores = spool.tile([P, ST, H], f16, tag="scores")
                with nc.allow_low_precision(
                    reason="fp16 scores: |s|<30; bf16 folds avg out"
                ):
                    nc.vector.reduce_sum(
                        scores, f32t, axis=mybir.AxisListType.X,
                    )
                E_sup = spool.tile([P, ST, H], bf16, tag="esup")
                nc.scalar.activation(
                    out=E_sup.rearrange("p t h -> p (t h)"),
                    in_=scores.rearrange("p t h -> p (t h)"),
                    func=mybir.ActivationFunctionType.Exp,
                )
                for t in range(ST):
                    first_ = first and t == 0
                    last_ = last and t == ST - 1
                    e_sl = E_sup[:, t, :]
                    f_sl = fb[:, lo + t, :]
                    nc.tensor.matmul(
                        sumE2[:, b : b + 1], e_sl, ones_sb,
                        start=first_, stop=last_,
                    )
                    nc.tensor.matmul(
                        uwfA[0:8, :], e_sl, f_sl[:, 0:512],
                        start=first_, stop=last_,
                    )
                    nc.tensor.matmul(
                        uwfB[0:8, :], e_sl, f_sl[:, 512:1024],
                        start=first_, stop=last_,
                    )

            def phase_scores(b):
                qk_bc3 = qk_bcs[b].rearrange(
                    "p (o c) -> p o c", o=1
                ).broadcast_to([P, ST, C])
                uwfA = ps_uwf.tile([P, 512], f32, tag="uwfA")
                uwfB = ps_uwf.tile([P, 512], f32, tag="uwfB")
                uwfs[b] = (uwfA, uwfB)
                pool_tmps = {}
                for st in POOL_MULS[b]:
                    pool_tmps[st] = pool_mul_tile(b, st)
                dve_tiles = [st for st in range(SUP) if st not in POOL_MULS[b]]
                order = dve_tiles + list(POOL_MULS[b])
                for i, st in enumerate(order):
                    scores_super_tile(
                        b, st, qk_bc3, uwfA, uwfB,
                        first=(i == 0), last=(i == len(order) - 1),
                        pool_tmp=pool_tmps.get(st),
                    )

            def tail_recip(b):
                recip = small.tile([8, 1], f32, tag="recip")
                nc.vector.reciprocal(recip, sumE2[:, b : b + 1])
                recips[b] = recip

            def tail_ctx(b):
                """ctx8 (+bv) -> broadcast bf16 SBUF tile.  ACT/PE only."""
                uwfA, uwfB = uwfs[b]
                recip = recips[b]
                # PSUM -> SBUF with the 1/sumE row scale fused into the copy
                uwf_sb = small.tile([8, C], f32, tag="uwfsb", bufs=1)
                nc.scalar.activation(
                    out=uwf_sb[:, 0:512], in_=uwfA[0:8, :],
                    func=mybir.ActivationFunctionType.Copy, scale=recip,
                )
                nc.scalar.activation(
                    out=uwf_sb[:, 512:1024], in_=uwfB[0:8, :],
                    func=mybir.ActivationFunctionType.Copy, scale=recip,
                )
                # per-head PE transpose; group h's diagonal column sits at
                # col 10*h (stride 10) given the h*9 packing below
                wfT8_ps = ps_tail.tile([P, H * 10], f32, tag="wft8")
                for h in range(H):
                    nc.tensor.transpose(
                        wfT8_ps[:, h * 9 : h * 9 + H],
                        uwf_sb[:, h * HD : (h + 1) * HD],
                        id8_sb,
                    )
                wfd_sb = small.tile([P, H], bf16, tag="wfd", bufs=1)
                nc.scalar.copy(
                    out=wfd_sb,
                    in_=wfT8_ps.rearrange("p (h n) -> p h n", n=10)[:, :, 0],
                )
                # ctx row per 512-half: bv seeded via K=1 ones ⊗ bv, then
                # 4 per-head wfd . WvT accumulate matmuls; broadcast down
                # partitions (ones ⊗ ctx_row) and copy out per half.
                ctx_bc = small.tile([P, C], bf16, tag="ctxbc")
                if POOL_RESID[b]:
                    ctx_bcp = small.tile([P, C], bf16, tag="ctxbcp")
                else:
                    ctx_bcp = None
                for half in range(2):
                    cs = slice(half * 512, (half + 1) * 512)
                    ctx_ps = ps_tail.tile([1, 512], f32, tag="ctxrow")
                    nc.tensor.matmul(
                        ctx_ps, one1_sb, bvf_sb[:, cs],
                        start=True, stop=False, skip_group_check=True,
                    )
                    for hh in range(4):
                        h = half * 4 + hh
                        nc.tensor.matmul(
                            ctx_ps[0:1, hh * HD : (hh + 1) * HD],
                            wfd_sb[:, h : h + 1],
                            wvt_sb,
                            start=False,
                            stop=(hh == 3),
                            skip_group_check=True,
                        )
                    ctx_row = small.tile([1, 512], bf16, tag="ctxrowsb")
                    nc.scalar.copy(out=ctx_row, in_=ctx_ps)
                    ctx_bc_ps = ps_tail.tile([P, 512], f32, tag="ctxbcps")
                    nc.tensor.matmul(
                        ctx_bc_ps, onesrow_sb, ctx_row,
                        start=True, stop=True,
                    )
                    nc.scalar.copy(out=ctx_bc[:, cs], in_=ctx_bc_ps)
                    if ctx_bcp is not None:
                        nc.scalar.copy(out=ctx_bcp[:, cs], in_=ctx_bc_ps)
                ctxs[b] = (ctx_bc, ctx_bcp)

            def resid_pool(b):
                """Pool-engine adds + scalar-ring stores for its tiles."""
                fb = fbs[b]
                ctx_bc2 = ctxs[b][1].rearrange("p (o c) -> p o c", o=1).broadcast_to(
                    [P, ST, C]
                )
                oview = out_t[b].rearrange("(st p t) c -> st p t c", p=P, t=ST)
                for st in POOL_RESID[b]:
                    lo = st * ST
                    ost = ostagep.tile([P, ST, C], bf16, tag="oslp")
                    nc.gpsimd.tensor_add(ost, fb[:, lo : lo + ST, :], ctx_bc2)
                    nc.scalar.dma_start(out=oview[st], in_=ost)

            def resid_dve(b):
                """DVE adds two super tiles per instruction + sync stores."""
                fb = fbs[b]
                ctx_bc4 = ctxs[b][0].rearrange("p (o c) -> p o c", o=1).broadcast_to(
                    [P, 2 * ST, C]
                )
                ctx_bc2 = ctxs[b][0].rearrange("p (o c) -> p o c", o=1).broadcast_to(
                    [P, ST, C]
                )
                oview = out_t[b].rearrange("(st p t) c -> st p t c", p=P, t=ST)
                tiles = [st for st in range(SUP) if st not in POOL_RESID[b]]
                i = 0
                while i < len(tiles):
                    if i + 1 < len(tiles) and tiles[i + 1] == tiles[i] + 1:
                        st = tiles[i]
                        lo = st * ST
                        ost = ostage.tile([P, 2 * ST, C], bf16, tag="osl")
                        nc.vector.tensor_add(
                            ost, fb[:, lo : lo + 2 * ST, :], ctx_bc4
                        )
                        nc.sync.dma_start(out=oview[st], in_=ost[:, 0:ST, :])
                        nc.sync.dma_start(out=oview[st + 1], in_=ost[:, ST : 2 * ST, :])
                        i += 2
                    else:
                        st = tiles[i]
                        lo = st * ST
                        ost = ostage.tile([P, ST, C], bf16, tag="osl1")
                        nc.vector.tensor_add(ost, fb[:, lo : lo + ST, :], ctx_bc2)
                        nc.sync.dma_start(out=oview[st], in_=ost)
                        i += 1

            phase_scores(0)
            tail_recip(0)
            tail_ctx(0)
            phase_scores(1)
            tail_recip(1)
            tail_ctx(1)
            resid_pool(0)
            resid_dve(0)
            resid_pool(1)
            resid_dve(1)

    nc.finalize()
    return nc


def _get_program():
    if "nc" not in _CACHE:
        _CACHE["nc"] = _build_program()
    return _CACHE["nc"]


def _prep_in_maps(features, preference, Wq, bq, Wk, Wv, bv):
    import ml_dtypes

    f32 = np.float32
    bf16 = ml_dtypes.bfloat16
    # qk[b,h,:] = (pref[b,h]*Wq[:,0] + bq) @ Wk   -> flat [B, C]
    q = preference[:, :, None] * Wq[:, 0][None, None, :] + bq  # [B,H,HD]
    qk = np.einsum("bhe,ed->bhd", q, Wk)  # [B,H,HD]
    qkflat = np.ascontiguousarray(qk.reshape(B, C)).astype(bf16)
    wvt = np.ascontiguousarray(Wv.T).astype(bf16)
    bvflat = np.ascontiguousarray(np.tile(bv, H)[None, :]).astype(bf16)
    id8 = np.eye(8, dtype=f32)
    fbf = np.ascontiguousarray(features).astype(bf16)

    in_maps = []
    for i in range(N_CORES):
        sl = slice(i * BPC, (i + 1) * BPC)
        in_maps.append(
            {
                "features": fbf[sl],
                "qkflat": qkflat[sl],
                "wvt": wvt,
                "bvflat": bvflat,
                "ident8": id8,
            }
        )
    return in_maps


def kernel(features, preference, Wq, bq, Wk, bk, Wv, bv, **_ignored):
    features = np.asarray(features, dtype=np.float32)
    preference = np.asarray(preference, dtype=np.float32)
    Wq = np.asarray(Wq, dtype=np.float32)
    bq = np.asarray(bq, dtype=np.float32)
    Wk = np.asarray(Wk, dtype=np.float32)
    Wv = np.asarray(Wv, dtype=np.float32)
    bv = np.asarray(bv, dtype=np.float32)

    from concourse.bass_utils import run_bass_kernel_spmd

    nc = _get_program()
    in_maps = _prep_in_maps(features, preference, Wq, bq, Wk, Wv, bv)
    res = run_bass_kernel_spmd(nc, in_maps, core_ids=list(range(N_CORES)))
    out = np.concatenate([r["out"] for r in res.results], axis=0)
    return out.astype(np.float32)
